# revision 1
# baseline (speedup 1.0000x reference)
"""Trainium2 Bass kernel for bag-level attention (ragged_sequence).

Math (per bag b over its 16 sentences i):
    att_i  = <x_i, rel[q_i]>
    w      = softmax(att) within bag
    logits = (sum_i w_i x_i) @ rel.T + bias

Key identity: logits[b] = sum_i w_i S[i,:] + bias with S = x @ rel.T, so x is
read from HBM exactly once.

Precision: x and rel are split on the host into fp16 hi + fp16 lo
(x = hi + lo, 22-bit combined mantissa). The four partial products
hi*hi + hi*lo + lo*hi + lo*lo are accumulated in fp32 PSUM, reproducing fp32
accuracy (~1e-6 rel) while running the TensorE at full fp16 rate (fp32
matmuls run at quarter rate and do not warm the HAM clock gate).

Device layout (per core, rows = N/8 sentences):
    S.T split over two partition blocks of PSUM st[128, ch]:
      rows 0:64   = relT_hi(64-col zero-padded).T @ xT_{hi,lo}   (tile_position (0,0))
      rows 64:128 = relT_lo(padded).T @ xT_{hi,lo}               (tile_position (0,64))
    The two col-tiles share each moving stream (concurrent sub-array execution).
    att  = partition_all_reduce(st * onehot2)        (GpSimd; onehot2 has the
           one-hot replicated in both blocks, built on host)
    e    = exp(att)                                  (ScalarE)
    ebs  = partition_broadcast(e)                    (GpSimd)
    lu[128, bags] = windowed reduce_16(st * ebs)     (VectorE)
    logitsU.T[53, bags] = stacked_identity.T @ lu    (recombines hi+lo blocks)
    * 1/z, + bias, final PE transpose to [bags, 53].
"""

import os
from contextlib import ExitStack

import numpy as np

import concourse.bass as bass
import concourse.tile as tile
from concourse import bacc, library_config, mybir
from concourse.bass_utils import run_bass_kernel_spmd

# Problem constants (hardcoded per spec nn_Attention_85478439125349)
N = 262144
B = 16384
D = 768
C = 53
BAG = 16
N_CORES = 8
ROWS = N // N_CORES          # 32768 sentences per core
BAGS = B // N_CORES          # 2048 bags per core
KCH = D // 128               # 6 contraction chunks
F32 = mybir.dt.float32
F16 = mybir.dt.float16


def build_nc(rows: int, sc: int = 1024, ch: int = 512) -> bass.Bass:
    """Build the per-core Bass program for `rows` sentences (bags of BAG)."""
    assert rows % sc == 0 and sc % ch == 0 and ch % BAG == 0
    bags = rows // BAG
    n_sc = rows // sc          # superchunks (DMA granularity)
    n_ch = sc // ch            # compute chunks per superchunk
    chb = ch // BAG            # bags per compute chunk (32)
    scb = sc // BAG            # bags per superchunk (128)

    nc = bacc.Bacc()
    # x hi/lo fp16, partition-major packed per superchunk so each partition's
    # DMA run is KCH*sc contiguous elements: xt4[h][p, isc, k, j] =
    # xT_h[128k+p, isc*sc+j]
    xt4h = nc.declare_dram_parameter(
        "xt4h", [128, rows // sc, KCH, sc], F16, isOutput=False
    )
    xt4l = nc.declare_dram_parameter(
        "xt4l", [128, rows // sc, KCH, sc], F16, isOutput=False
    )
    # one-hot mask replicated into both partition blocks: [128, rows]
    oht = nc.declare_dram_parameter("oht", [128, rows], F16, isOutput=False)
    # relT hi/lo, each zero-padded to 64 output columns: [D, 2, 64]
    relt2 = nc.declare_dram_parameter("relt2", [D, 2, 64], F16, isOutput=False)
    # stacked identity [128, C]: row k -> col m if k==m or k==64+m
    sident = nc.declare_dram_parameter("sident", [128, C], F32, isOutput=False)
    identm = nc.declare_dram_parameter("identm", [C, C], F32, isOutput=False)
    biast = nc.declare_dram_parameter("biast", [C, 1], F32, isOutput=False)
    out = nc.declare_dram_parameter("out", [bags, C], F32, isOutput=True)

    relt_v = relt2.rearrange("(k p) h c -> k p h c", p=128)  # [KCH, 128, 2, 64]

    with tile.TileContext(nc) as tc, ExitStack() as ctx:
        consts = ctx.enter_context(tc.tile_pool(name="consts", bufs=1))
        xpool = ctx.enter_context(tc.tile_pool(name="xpool", bufs=2))
        ohpool = ctx.enter_context(tc.tile_pool(name="ohpool", bufs=2))
        work = ctx.enter_context(tc.tile_pool(name="work", bufs=3))
        psum = ctx.enter_context(tc.tile_pool(name="psum", bufs=2, space="PSUM"))

        # --- constants ---
        relt_sb = consts.tile([128, KCH, 2, 64], F16)
        nc.sync.dma_start(out=relt_sb, in_=relt_v.transpose([1, 0, 2, 3]))
        sident_sb = consts.tile([128, C], F32)
        nc.sync.dma_start(out=sident_sb, in_=sident[:, :])
        bias_sb = consts.tile([C, 1], F32)
        nc.sync.dma_start(out=bias_sb, in_=biast[:, :])
        ident = consts.tile([C, C], F32)
        nc.sync.dma_start(out=ident, in_=identm[:, :])
        zeros_sb = consts.tile([64, 512], F32)
        nc.vector.memset(zeros_sb, 0.0)
        ones128 = consts.tile([128, 1], F32)
        nc.vector.memset(ones128, 1.0)
        nc.gpsimd.load_library(library_config.attn)
        # accumulator for logits^T [C, bags] and staging for transposed output
        lt_acc = consts.tile([C, bags], F32)
        logits_sb = consts.tile([128, bags // 128, C], F32)

        # Software-pipelined chunk loop: per-engine instruction streams are
        # in-order, so chunk i's late stage (which waits on the GpSimd/ACT
        # softmax chain) is emitted only after chunk i+1's early stages —
        # otherwise VectorE blocks on w(i) before issuing sm(i+1) and the
        # whole chain serializes.
        n_total = n_sc * n_ch
        pend_a = {}  # chunk -> (st, sm): waiting for att/exp/bcast stage
        pend_b = {}  # chunk -> (st, ebs): waiting for weighted-sum stage

        def stage_mid(i):
            # att = column sums of sm via fp32 ones-matmul; exp; broadcast
            st, sm = pend_a.pop(i)
            att = psum.tile([1, ch], F32, tag="att", bufs=2)
            nc.tensor.matmul(att, lhsT=ones128, rhs=sm)
            e = work.tile([1, ch], F32, tag="e")
            nc.scalar.activation(e, att, mybir.ActivationFunctionType.Exp)
            ebs = work.tile([128, ch], F32, tag="ebs")
            nc.gpsimd.partition_broadcast(ebs, e, channels=128)
            pend_b[i] = (st, ebs)

        def stage_late(i):
            st, ebs = pend_b.pop(i)
            w = work.tile([128, ch], F32, tag="w")
            nc.vector.tensor_mul(w, st, ebs)
            lu = work.tile([128, chb], F32, tag="lu")
            nc.vector.reduce_sum(
                lu, w.rearrange("p (b j) -> p b j", j=BAG), axis=mybir.AxisListType.X
            )
            # recombine hi+lo partition blocks: [53, chb]
            lc = psum.tile([C, chb], F32, tag="lc")
            nc.tensor.matmul(lc, lhsT=sident_sb, rhs=lu)
            # z per bag from the broadcast copy; normalize + bias
            zb = work.tile([C, chb], F32, tag="zb")
            nc.vector.reduce_sum(
                zb,
                ebs[0:C, :].rearrange("p (b j) -> p b j", j=BAG),
                axis=mybir.AxisListType.X,
            )
            rzb = work.tile([C, chb], F32, tag="rzb")
            nc.vector.reciprocal(rzb, zb)
            ob = i * chb
            nc.vector.tensor_mul(lt_acc[:, ob : ob + chb], lc, rzb)
            nc.vector.tensor_scalar_add(
                out=lt_acc[:, ob : ob + chb],
                in0=lt_acc[:, ob : ob + chb],
                scalar1=bias_sb,
            )
            # once a 128-bag block is complete, transpose it to [bags, C]
            # (overlaps with the remaining chunks instead of a serial tail)
            if (i + 1) * chb % 128 == 0:
                t = ((i + 1) * chb) // 128 - 1
                pt = psum.tile([128, C], F32, tag="att", bufs=2)
                nc.tensor.transpose(pt, lt_acc[:, t * 128 : (t + 1) * 128], ident)
                nc.vector.tensor_copy(logits_sb[:, t, :], pt)

        x_sb = oh_sb = None
        for i in range(n_total):
            isc, ic = divmod(i, n_ch)
            if ic == 0:
                x_sb = xpool.tile([128, KCH, 2, sc], F16, bufs=3)
                nc.sync.dma_start(out=x_sb[:, :, 0, :], in_=xt4h[:, isc, :, :])
                nc.sync.dma_start(out=x_sb[:, :, 1, :], in_=xt4l[:, isc, :, :])
                oh_sb = ohpool.tile([128, sc], F16, bufs=3)
                nc.sync.dma_start(out=oh_sb, in_=oht[:, isc * sc : (isc + 1) * sc])

            cs = slice(ic * ch, (ic + 1) * ch)
            st = psum.tile([128, ch], F32, tag="st", bufs=4)
            # Zero the lo block: its matmuls use start=False (a second
            # bank-wide has_written clear would wipe the hi block), so on
            # sim/stale PSUM the first accumulate needs zeroed ground.
            nc.vector.memset(st[64:128, :], 0.0)
            for k in range(KCH):
                # hi block: r_hi against both x streams
                for h in range(2):
                    nc.tensor.matmul(
                        st[0:64, :],
                        lhsT=relt_sb[:, k, 0, :],
                        rhs=x_sb[:, k, h, cs],
                        start=(k == 0 and h == 0),
                        stop=(k == KCH - 1 and h == 1),
                        tile_position=(0, 0),
                    )
                # lo block: r_lo against x_hi only (lo*lo term ~2^-22, dropped)
                nc.tensor.matmul(
                    st[64:128, :],
                    lhsT=relt_sb[:, k, 1, :],
                    rhs=x_sb[:, k, 0, cs],
                    start=False,
                    stop=False,
                    skip_group_check=True,
                    tile_position=(0, 64),
                )
            sm = work.tile([128, ch], F32, tag="sm")
            nc.vector.tensor_mul(sm, st, oh_sb[:, cs])
            pend_a[i] = (st, sm)
            if i > 0:
                stage_mid(i - 1)
            if i > 1:
                stage_late(i - 2)
        stage_mid(n_total - 1)
        stage_late(n_total - 2)
        stage_late(n_total - 1)
        nc.sync.dma_start(
            out=out.rearrange("(t p) c -> p t c", p=128), in_=logits_sb
        )
    return nc


_NC_CACHE: dict = {}


def _get_nc(rows: int) -> bass.Bass:
    if rows not in _NC_CACHE:
        nc = build_nc(rows)
        nc.finalize()
        _NC_CACHE[rows] = nc
    return _NC_CACHE[rows]


def _numpy_fallback(x, rel_weight, bias, input_scope, query):
    """Pure-numpy replication of the reference for non-uniform bag layouts."""
    n = x.shape[0]
    num_bags = input_scope.shape[0] - 1
    seg = np.searchsorted(input_scope[1:], np.arange(n), side="right")
    att = np.einsum("nd,nd->n", x, rel_weight[query]).astype(np.float32)
    valid = seg < num_bags
    segv = seg[valid]
    attv = att[valid]
    m = np.full(num_bags, -np.inf, dtype=np.float32)
    np.maximum.at(m, segv, attv)
    e = np.zeros(n, dtype=np.float32)
    e[valid] = np.exp(attv - m[segv])
    z = np.zeros(num_bags, dtype=np.float32)
    np.add.at(z, segv, e[valid])
    w = np.zeros(n, dtype=np.float32)
    nz = z[segv] != 0
    w_valid = np.zeros(segv.shape[0], dtype=np.float32)
    w_valid[nz] = e[valid][nz] / z[segv][nz]
    w[valid] = w_valid
    repre = np.zeros((num_bags, x.shape[1]), dtype=np.float32)
    np.add.at(repre, segv, (x[valid] * w[valid][:, None]).astype(np.float32))
    return repre @ rel_weight.T + bias


def _split_f16(a):
    hi = a.astype(np.float16)
    lo = (a - hi.astype(np.float32)).astype(np.float16)
    return hi, lo


def _pack_x(xt_h, sc):
    """[D, rows] -> [128, rows//sc, KCH, sc] so each partition's per-superchunk
    DMA run (KCH*sc elements) is contiguous."""
    rows = xt_h.shape[1]
    v = xt_h.reshape(KCH, 128, rows // sc, sc)
    return np.ascontiguousarray(v.transpose(1, 2, 0, 3))


def _prepare_in_maps(x, rel_weight, bias, query, sc=1024):
    rh, rl = _split_f16(rel_weight)  # [C, D] each
    relt2 = np.zeros((D, 2, 64), dtype=np.float16)
    relt2[:, 0, :C] = rh.T
    relt2[:, 1, :C] = rl.T
    sident = np.zeros((128, C), dtype=np.float32)
    sident[np.arange(C), np.arange(C)] = 1.0
    sident[64 + np.arange(C), np.arange(C)] = 1.0
    identm = np.eye(C, dtype=np.float32)
    biast = np.ascontiguousarray(bias.reshape(C, 1)).astype(np.float32)
    q = query.astype(np.int64)
    in_maps = []
    for c in range(N_CORES):
        lo_r, hi_r = c * ROWS, (c + 1) * ROWS
        xh, xl = _split_f16(x[lo_r:hi_r])
        oh = np.zeros((128, ROWS), dtype=np.float16)
        qc = q[lo_r:hi_r]
        ar = np.arange(ROWS)
        oh[qc, ar] = 1.0
        oh[64 + qc, ar] = 1.0
        in_maps.append(
            {"xt4h": _pack_x(xh.T, sc), "xt4l": _pack_x(xl.T, sc), "oht": oh,
             "relt2": relt2, "sident": sident, "identm": identm,
             "biast": biast}
        )
    return in_maps


def run_device(x, rel_weight, bias, query, trace=False, **kwargs):
    nc = _get_nc(ROWS)
    in_maps = _prepare_in_maps(x, rel_weight, bias, query)
    res = run_bass_kernel_spmd(
        nc, in_maps, core_ids=list(range(N_CORES)), trace=trace, **kwargs
    )
    outs = [np.asarray(r["out"]) for r in res.results]
    return np.concatenate(outs, axis=0), res


def kernel(x, rel_weight, bias, input_scope, query):
    x = np.asarray(x, dtype=np.float32)
    rel_weight = np.asarray(rel_weight, dtype=np.float32)
    bias = np.asarray(bias, dtype=np.float32)
    input_scope = np.asarray(input_scope)
    query = np.asarray(query)

    expected_scope = np.arange(B + 1, dtype=np.int64) * (N // B)
    if (
        x.shape == (N, D)
        and rel_weight.shape == (C, D)
        and input_scope.shape == (B + 1,)
        and np.array_equal(input_scope.astype(np.int64), expected_scope)
    ):
        out, _ = run_device(x, rel_weight, bias, query)
        return out
    return _numpy_fallback(x, rel_weight, bias, input_scope, query)



# revision 6
# speedup vs baseline: 2.1641x; 2.1641x over previous
"""Trainium2 Bass kernel for bag-level attention (ragged_sequence).

Math (per bag b over its 16 sentences i):
    att_i  = <x_i, rel[q_i]>
    w      = softmax(att) within bag
    logits = (sum_i w_i x_i) @ rel.T + bias

Key identity: logits[b] = sum_i w_i S[i,:] + bias with S = x @ rel.T, so x is
read from HBM exactly once.

v2 design (vs the hi/lo-fp16 baseline):
  * x is plain fp16 (the 2e-2 correctness gate leaves ~50x headroom at fp16;
    measured 3.5e-4). Halves HBM traffic - the kernel is DMA-bound.
  * 2-up partition packing: each compute step handles TWO 512-sentence
    chunks, A in PSUM partitions 0:64 (tile_position (0,0)) and B in 64:128
    (tile_position (0,64)).  Halves per-sentence VectorE/GpSimd cost.
  * z-row trick: st rows 53 (A) and 117 (B) are forced to 1.0 (they are
    zero-padded rel rows), so the per-bag weighted reduce lu = reduce16(st*ebs)
    yields z_b = sum(e) in class-row 53 for free; z then flows with the
    logits through the lc matmul and PE transpose, and the normalize+bias
    happens post-transpose as cheap [128, 53] ops (bags on partitions).
  * x packed so each partition's per-superchunk DMA run is KCH*sc*2B = 24KB
    contiguous (few, large descriptors; the baseline's 2KB descriptors were
    descriptor-rate-bound).
  * one-hot mask stored as fp8 (exact for 0/1), 1B per sentence per 128 rows.
"""

import os
from contextlib import ExitStack

import numpy as np
import ml_dtypes

import concourse.bass as bass
import concourse.tile as tile
from concourse import bacc, library_config, mybir
from concourse.bass_utils import run_bass_kernel_spmd

# Problem constants (hardcoded per spec nn_Attention_85478439125349)
N = 262144
B = 16384
D = 768
C = 53
BAG = 16
N_CORES = 8
ROWS = N // N_CORES          # 32768 sentences per core
BAGS = B // N_CORES          # 2048 bags per core
KCH = D // 128               # 6 contraction chunks
F32 = mybir.dt.float32
F16 = mybir.dt.float16
F8 = mybir.dt.float8e4

# x in fp8-e3m4 (1 B/elem HBM traffic, predicted metric 1.66e-2 vs the 2e-2
# gate) or fp16 (2 B/elem, metric 3.5e-4). Flip X_FP8 to trade accuracy
# margin for DMA time.
X_FP8 = False
X_DT = mybir.dt.float8e3 if X_FP8 else F16
X_NP = ml_dtypes.float8_e3m4 if X_FP8 else np.float16

CH = 512                     # sentences per PSUM block per chunk
PAIR = 2 * CH                # sentences per compute chunk (2-up packing)
ZROW = C                     # class-row index used to carry z (53, a pad row)


def build_nc(rows: int, sc: int = 2048) -> bass.Bass:
    """Per-core Bass program for `rows` sentences (uniform bags of BAG)."""
    assert rows % sc == 0 and sc % PAIR == 0
    bags = rows // BAG
    n_sc = rows // sc          # superchunks (DMA granularity)
    n_ch = sc // PAIR          # compute chunks per superchunk
    n_total = n_sc * n_ch
    pb = PAIR // BAG           # bags per compute chunk (64): 32 A + 32 B
    assert bags % 128 == 0

    nc = bacc.Bacc()
    # x fp16, packed so partition p's superchunk run is contiguous:
    # xt4[p, isc, k, j] = xT[128k+p, isc*sc + j]
    xt4 = nc.declare_dram_parameter("xt4", [128, n_sc, KCH, sc], X_DT, isOutput=False)
    # one-hot mask, 2-up layout: rows 0:53 chunk-A one-hot, 64:117 chunk-B
    oht = nc.declare_dram_parameter("oht", [128, rows // 2], F8, isOutput=False)
    # rel.T zero-padded to 64 cols: relt[p, k, c] = rel[c, 128k+p]
    relt = nc.declare_dram_parameter("relt", [128, KCH, 64], F16, isOutput=False)
    # lc stationaries: sidentA rows 0:54 -> cols 0:54, sidentB rows 64:118
    sidentA = nc.declare_dram_parameter("sidentA", [128, C + 1], F32, isOutput=False)
    sidentB = nc.declare_dram_parameter("sidentB", [128, C + 1], F32, isOutput=False)
    identm = nc.declare_dram_parameter("identm", [C + 1, C + 1], F32, isOutput=False)
    # att stationaries: onesAB[:, 0] = ones on partitions 0:64 (A),
    # onesAB[:, 1] = ones on partitions 64:128 (B).  Used as two separate
    # M=1 matmuls so each att row lands at partition 0 of its own PSUM bank
    # (HW partition_broadcast only reads/writes partition-0-based APs).
    onesAB = nc.declare_dram_parameter("onesAB", [128, 2], F16, isOutput=False)
    biasrow = nc.declare_dram_parameter("biasrow", [128, C], F32, isOutput=False)
    # z-row stationary: one matmul col (21) set so an accumulating matmul
    # adds +1.0 to partition 32+21=53 (or 96+21=117) of st
    zsel = nc.declare_dram_parameter("zsel", [1, 32], F16, isOutput=False)
    # output partition-major: out[p, t, c] = logits[128t + p, c]
    out = nc.declare_dram_parameter("out", [128, bags // 128, C], F32, isOutput=True)

    with tile.TileContext(nc) as tc, ExitStack() as ctx:
        consts = ctx.enter_context(tc.tile_pool(name="consts", bufs=1))
        xpool = ctx.enter_context(tc.tile_pool(name="xpool", bufs=3))
        ohpool = ctx.enter_context(tc.tile_pool(name="ohpool", bufs=3))
        work = ctx.enter_context(tc.tile_pool(name="work", bufs=3))
        psum = ctx.enter_context(tc.tile_pool(name="psum", bufs=1, space="PSUM"))

        # --- constants ---
        relt_sb = consts.tile([128, KCH, 64], F16)
        nc.sync.dma_start(out=relt_sb, in_=relt[:, :, :])
        sidA_sb = consts.tile([128, C + 1], F32)
        nc.sync.dma_start(out=sidA_sb, in_=sidentA[:, :])
        sidB_sb = consts.tile([128, C + 1], F32)
        nc.sync.dma_start(out=sidB_sb, in_=sidentB[:, :])
        ident = consts.tile([C + 1, C + 1], F32)
        nc.sync.dma_start(out=ident, in_=identm[:, :])
        onesAB_sb = consts.tile([128, 2], F16)
        nc.sync.dma_start(out=onesAB_sb, in_=onesAB[:, :])
        bias_sb = consts.tile([128, C], F32)
        nc.sync.dma_start(out=bias_sb, in_=biasrow[:, :])
        onesrow = consts.tile([1, CH], F16)
        nc.vector.memset(onesrow, 1.0)
        zsel_sb = consts.tile([1, 32], F16)
        nc.sync.dma_start(out=zsel_sb, in_=zsel[:, :])
        nc.gpsimd.load_library(library_config.attn)
        # logits^T accumulator [54, bags] and transposed staging [128, ., 54]
        lt_acc = consts.tile([C + 1, bags], F32)
        logits_sb = consts.tile([128, bags // 128, C], F32)

        # Software-pipelined chunk loop (engine queues are in-order; chunk
        # i's late stages are emitted after chunk i+1's early stages so no
        # engine blocks on the att->exp->broadcast chain).
        pend_m = {}  # chunk -> (st, sm): waiting for att/exp/broadcast
        pend_l = {}  # chunk -> (st, ebs): waiting for weighted-sum stage

        def stage_mid(i):
            st, sm = pend_m.pop(i)
            attA = psum.tile([1, CH], F32, tag="attA", bufs=1)
            nc.tensor.matmul(attA, lhsT=onesAB_sb[:, 0:1], rhs=sm)
            attB = psum.tile([1, CH], F32, tag="attB", bufs=1)
            nc.tensor.matmul(attB, lhsT=onesAB_sb[:, 1:2], rhs=sm)
            eA = work.tile([1, CH], F16, tag="eA")
            nc.scalar.activation(eA, attA, mybir.ActivationFunctionType.Exp)
            eB = work.tile([1, CH], F16, tag="eB")
            nc.scalar.activation(eB, attB, mybir.ActivationFunctionType.Exp)
            # HW partition_broadcast ignores out-slice partition offsets, so
            # broadcast into full tiles: A needs only rows 0:64, B needs a
            # full 128 so rows 64:128 are valid for the lane-aligned mul.
            ebsA = work.tile([64, CH], F16, tag="ebsA")
            nc.gpsimd.partition_broadcast(ebsA, eA, channels=64)
            ebsB = work.tile([128, CH], F16, tag="ebsB")
            nc.gpsimd.partition_broadcast(ebsB, eB, channels=128)
            pend_l[i] = (st, ebsA, ebsB)

        def stage_late(i):
            st, ebsA, ebsB = pend_l.pop(i)
            w = work.tile([128, CH], F16, tag="w")
            nc.vector.tensor_mul(w[0:64, :], st[0:64, :], ebsA)
            nc.vector.tensor_mul(w[64:128, :], st[64:128, :], ebsB[64:128, :])
            lu = work.tile([128, pb // 2], F32, tag="lu")
            nc.vector.reduce_sum(
                lu, w.rearrange("p (b j) -> p b j", j=BAG), axis=mybir.AxisListType.X
            )
            lcA = psum.tile([C + 1, pb // 2], F32, tag="lcA", bufs=1)
            nc.tensor.matmul(lcA, lhsT=sidA_sb, rhs=lu)
            lcB = psum.tile([C + 1, pb // 2], F32, tag="lcB", bufs=1)
            nc.tensor.matmul(lcB, lhsT=sidB_sb, rhs=lu)
            ob = i * pb
            nc.scalar.copy(lt_acc[:, ob : ob + 32], lcA)
            nc.scalar.copy(lt_acc[:, ob + 32 : ob + 64], lcB)
            # every 128 accumulated bags: transpose, normalize, add bias
            if (i + 1) * pb % 128 == 0:
                t = ((i + 1) * pb) // 128 - 1
                pt = psum.tile([128, C + 1], F32, tag="pt", bufs=1)
                nc.tensor.transpose(pt, lt_acc[:, t * 128 : (t + 1) * 128], ident)
                rz = work.tile([128, 1], F32, tag="rz")
                nc.vector.reciprocal(rz, pt[:, ZROW : ZROW + 1])
                nc.vector.scalar_tensor_tensor(
                    out=logits_sb[:, t, :],
                    in0=pt[:, 0:C],
                    scalar=rz,
                    in1=bias_sb,
                    op0=mybir.AluOpType.mult,
                    op1=mybir.AluOpType.add,
                )

        x_sb = oh_sb = None
        for i in range(n_total):
            isc, ic = divmod(i, n_ch)
            if ic == 0:
                x_sb = xpool.tile([128, KCH, sc], X_DT, bufs=3)
                nc.sync.dma_start(out=x_sb, in_=xt4[:, isc, :, :])
                oh_sb = ohpool.tile([128, sc // 2], F8, bufs=3)
                nc.sync.dma_start(
                    out=oh_sb, in_=oht[:, isc * (sc // 2) : (isc + 1) * (sc // 2)]
                )

            ca = slice(ic * PAIR, ic * PAIR + CH)          # chunk-A sentences
            cb = slice(ic * PAIR + CH, (ic + 1) * PAIR)    # chunk-B sentences
            st = psum.tile([128, CH], F32, tag="st", bufs=3)
            # block A: full accumulation group, then block B (its start=True
            # clears bank-wide has_written, which is safe once A is done)
            for k in range(KCH):
                nc.tensor.matmul(
                    st[0:64, :],
                    lhsT=relt_sb[:, k, :],
                    rhs=x_sb[:, k, ca],
                    start=(k == 0),
                    stop=(k == KCH - 1),
                    tile_position=(0, 0),
                )
            nc.tensor.matmul(
                st[32:64, :], lhsT=zsel_sb, rhs=onesrow,
                start=False, stop=False, skip_group_check=True,
                tile_position=(0, 32),
            )
            for k in range(KCH):
                nc.tensor.matmul(
                    st[64:128, :],
                    lhsT=relt_sb[:, k, :],
                    rhs=x_sb[:, k, cb],
                    start=(k == 0),
                    stop=(k == KCH - 1),
                    skip_group_check=True,
                    tile_position=(0, 64),
                )
            nc.tensor.matmul(
                st[96:128, :], lhsT=zsel_sb, rhs=onesrow,
                start=False, stop=False, skip_group_check=True,
                tile_position=(0, 96),
            )
            if i > 0:
                stage_mid(i - 1)
            sm = work.tile([128, CH], F16, tag="sm")
            oc = slice(ic * (PAIR // 2), (ic + 1) * (PAIR // 2))
            nc.vector.tensor_mul(sm, st, oh_sb[:, oc])
            pend_m[i] = (st, sm)
            if i > 1:
                stage_late(i - 2)
        stage_mid(n_total - 1)
        stage_late(n_total - 2)
        stage_late(n_total - 1)
        nc.sync.dma_start(out=out[:, :, :], in_=logits_sb)
    return nc


_NC_CACHE: dict = {}


def _get_nc(rows: int, sc: int = 2048) -> bass.Bass:
    key = (rows, sc)
    if key not in _NC_CACHE:
        nc = build_nc(rows, sc)
        nc.finalize()
        _NC_CACHE[key] = nc
    return _NC_CACHE[key]


def _numpy_fallback(x, rel_weight, bias, input_scope, query):
    """Pure-numpy replication of the reference for non-uniform bag layouts."""
    n = x.shape[0]
    num_bags = input_scope.shape[0] - 1
    seg = np.searchsorted(input_scope[1:], np.arange(n), side="right")
    att = np.einsum("nd,nd->n", x, rel_weight[query]).astype(np.float32)
    valid = seg < num_bags
    segv = seg[valid]
    attv = att[valid]
    m = np.full(num_bags, -np.inf, dtype=np.float32)
    np.maximum.at(m, segv, attv)
    e = np.zeros(n, dtype=np.float32)
    e[valid] = np.exp(attv - m[segv])
    z = np.zeros(num_bags, dtype=np.float32)
    np.add.at(z, segv, e[valid])
    w = np.zeros(n, dtype=np.float32)
    nz = z[segv] != 0
    w_valid = np.zeros(segv.shape[0], dtype=np.float32)
    w_valid[nz] = e[valid][nz] / z[segv][nz]
    w[valid] = w_valid
    repre = np.zeros((num_bags, x.shape[1]), dtype=np.float32)
    np.add.at(repre, segv, (x[valid] * w[valid][:, None]).astype(np.float32))
    return repre @ rel_weight.T + bias


def _pack_x(xc, sc):
    """x rows [rows, D] fp32 -> [128, rows//sc, KCH, sc] fp16 with each
    partition's per-superchunk run (KCH*sc elems) contiguous."""
    rows = xc.shape[0]
    v = xc.T.astype(X_NP).reshape(KCH, 128, rows // sc, sc)
    return np.ascontiguousarray(v.transpose(1, 2, 0, 3))


def _build_oh(qc):
    """One-hot mask in 2-up layout: [128, rows//2] fp8."""
    rows = qc.shape[0]
    s = np.arange(rows)
    col = (s // PAIR) * CH + (s % CH)
    rowoff = 64 * ((s % PAIR) // CH)
    oh = np.zeros((128, rows // 2), dtype=ml_dtypes.float8_e4m3)
    oh[rowoff + qc, col] = 1.0
    return oh


def _prepare_in_maps(x, rel_weight, bias, query, sc=2048):
    relt = np.zeros((128, KCH, 64), dtype=np.float16)
    relt[:, :, :C] = (
        rel_weight.T.astype(np.float16).reshape(KCH, 128, C).transpose(1, 0, 2)
    )
    sidA = np.zeros((128, C + 1), dtype=np.float32)
    sidA[np.arange(C + 1), np.arange(C + 1)] = 1.0
    sidB = np.zeros((128, C + 1), dtype=np.float32)
    sidB[64 + np.arange(C + 1), np.arange(C + 1)] = 1.0
    identm = np.eye(C + 1, dtype=np.float32)
    onesAB = np.zeros((128, 2), dtype=np.float16)
    onesAB[0:64, 0] = 1.0
    onesAB[64:128, 1] = 1.0
    biasrow = np.broadcast_to(bias.astype(np.float32), (128, C)).copy()
    zsel = np.zeros((1, 32), dtype=np.float16)
    zsel[0, ZROW - 32] = 1.0
    q = query.astype(np.int64)
    in_maps = []
    for c in range(N_CORES):
        lo_r, hi_r = c * ROWS, (c + 1) * ROWS
        in_maps.append(
            {
                "xt4": _pack_x(x[lo_r:hi_r], sc),
                "oht": _build_oh(q[lo_r:hi_r]),
                "relt": relt,
                "sidentA": sidA,
                "sidentB": sidB,
                "identm": identm,
                "onesAB": onesAB,
                "biasrow": biasrow,
                "zsel": zsel,
            }
        )
    return in_maps


def run_device(x, rel_weight, bias, query, trace=False, **kwargs):
    nc = _get_nc(ROWS)
    in_maps = _prepare_in_maps(x, rel_weight, bias, query)
    res = run_bass_kernel_spmd(
        nc, in_maps, core_ids=list(range(N_CORES)), trace=trace, **kwargs
    )
    outs = [
        np.asarray(r["out"]).transpose(1, 0, 2).reshape(BAGS, C)
        for r in res.results
    ]
    return np.concatenate(outs, axis=0), res


def kernel(x, rel_weight, bias, input_scope, query):
    x = np.asarray(x, dtype=np.float32)
    rel_weight = np.asarray(rel_weight, dtype=np.float32)
    bias = np.asarray(bias, dtype=np.float32)
    input_scope = np.asarray(input_scope)
    query = np.asarray(query)

    expected_scope = np.arange(B + 1, dtype=np.int64) * (N // B)
    if (
        x.shape == (N, D)
        and rel_weight.shape == (C, D)
        and input_scope.shape == (B + 1,)
        and np.array_equal(input_scope.astype(np.int64), expected_scope)
    ):
        out, _ = run_device(x, rel_weight, bias, query)
        return out
    return _numpy_fallback(x, rel_weight, bias, input_scope, query)
